# revision 5
# baseline (speedup 1.0000x reference)
"""VQ-VIB agent-communication kernel for Trainium2 (8 NeuronCores).

Reference computation (all fp32):
    mu      = hidden_state @ W_mu + b_mu                  # (64, 256)
    log_var = hidden_state @ W_var + b_var                # (64, 256)
    out     = eps * exp(0.5 * log_var) + mu               # (64, 256)
    token[n, d] = min_v (out[n, d] - vocab[v, d])**2      # min over 4096
    comm[j] = sum_{i != j} token[i] = (sum_i token[i]) - token[j]

Sharding: the 256 comm dims are split 32-per-core across 8 cores. Each core
gets the full hidden_state plus column slices of W_mu/W_var/b_*/eps/vocab,
computes its (64, 32) output slice independently (no collectives), and the
host concatenates along the dim axis.

Per-core layout for the VQ min: SBUF partitions p = 32*c + d hold local dim d
replicated over four agent-quarters c; the free axis is the full 4096-entry
vocab. For each group g of four agents (one per quarter, agent = 16*c + g):
  - ScalarE computes Square(vocab_T + (-out)) into bf16 for vocab[0:FDA]
  - VectorE folds it with a bf16 tensor_tensor min tree + final reduce_min
  - a custom fused DVE op handles vocab[FDA:4096] in fp32:
        accum_out = min(seed, min_v (vocab_T[v] - out)^2)
    seeded with the tree result, giving the final per-(agent, dim) min.
"""

from contextlib import ExitStack

import numpy as np

import concourse.bass as bass
import concourse.tile as tile
from concourse import bacc, mybir
from concourse.bass_utils import run_bass_kernel_spmd
from concourse.masks import make_identity

N_AGENTS = 64
HID = 1024
D = 256
V = 4096
NCORES = 8
DS = D // NCORES  # 32 dims per core

# Tuning knobs
USE_CUSTOM = True   # fused sq-diff+min-reduce DVE op for the tail of the vocab
FDA = 3328          # vocab elements handled by the ScalarE square + bf16 min tree
TREE_STOP_W = 416   # switch from tensor_tensor min tree to reduce_min at this width
SQ_BUFS = 3

f32 = mybir.dt.float32
bf16 = mybir.dt.bfloat16
ALU = mybir.AluOpType
ACTF = mybir.ActivationFunctionType

_CUSTOM_OP = None
_NC = None


def _sqdiff_min_op():
    """Register (idempotently) the fused DVE op:
        out = (in0 - s0)^2 ; accum_out = min(s1, min_free(out))
    """
    global _CUSTOM_OP
    if _CUSTOM_OP is not None:
        return _CUSTOM_OP
    import concourse.dve_ops as dve_ops
    from concourse.dve_spec import C0, C1, AluOp, Spec, Src0, lower, sq
    from concourse.dve_uop import DveOpSpec

    name = "SQDIFF_MIN_ANT"
    for op in dve_ops.OPS:
        if op.name == name:
            _CUSTOM_OP = op
            return op

    def ref(in0, in1, s0, s1, imm2):
        b = ((in0.astype(np.float32) - s0) ** 2).astype(np.float32)
        p = b.shape[0]
        acc = np.minimum(
            np.broadcast_to(np.asarray(s1, np.float32).reshape(-1, 1), (p, 1)),
            b.reshape(p, -1).min(axis=-1, keepdims=True),
        ).astype(np.float32)
        return b, acc

    spec = Spec(body=sq(Src0 - C0), accum=AluOp.MIN, accum_init=C1, reference=ref)
    shas = {}
    for ver in ("v3", "v4"):
        shas[ver] = DveOpSpec(name=name, opcode=1, uops=lower(spec, ver=ver)).sha(ver)
    op = dve_ops.DveOp(name, spec, subdim=False, uops_sha=shas)
    dve_ops.OPS.append(op)
    dve_ops._SUB_OPCODE_FOR_NAME[name] = dve_ops._CUSTOM_DVE_ROW_BASE + len(dve_ops.OPS) - 1
    assert dve_ops._SUB_OPCODE_FOR_NAME[name] < 0x20
    dve_ops.CUSTOM_DVE_SPECS[name] = spec
    _CUSTOM_OP = op
    return op


def build_nc():
    """Trace the per-core Bass program (identical on all 8 cores)."""
    if USE_CUSTOM:
        custom_op = _sqdiff_min_op()
    nc = bacc.Bacc(
        "TRN2", target_bir_lowering=False, debug=False, enable_asserts=False,
        num_devices=NCORES,
    )
    hs = nc.dram_tensor("hs", (N_AGENTS, HID), f32, kind="ExternalInput").ap()
    wmu = nc.dram_tensor("wmu", (HID, DS), f32, kind="ExternalInput").ap()
    wvar = nc.dram_tensor("wvar", (HID, DS), f32, kind="ExternalInput").ap()
    bmu = nc.dram_tensor("bmu", (DS, 1), f32, kind="ExternalInput").ap()
    bvar = nc.dram_tensor("bvar", (DS, 1), f32, kind="ExternalInput").ap()
    eps = nc.dram_tensor("eps", (N_AGENTS, DS), f32, kind="ExternalInput").ap()
    voc = nc.dram_tensor("voc", (V, DS), f32, kind="ExternalInput").ap()
    out = nc.dram_tensor("out", (N_AGENTS, DS), f32, kind="ExternalOutput").ap()

    with tile.TileContext(nc) as tc, ExitStack() as ctx:
        consts = ctx.enter_context(tc.tile_pool(name="consts", bufs=1))
        sqpool = ctx.enter_context(tc.tile_pool(name="sq", bufs=SQ_BUFS))
        tpool = ctx.enter_context(tc.tile_pool(name="tree", bufs=2))
        psT = ctx.enter_context(tc.tile_pool(name="psT", bufs=3, space="PSUM"))
        psMM = ctx.enter_context(tc.tile_pool(name="psMM", bufs=1, space="PSUM"))

        ident = consts.tile([128, 128], f32)
        make_identity(nc, ident)

        # ---- input loads -------------------------------------------------
        hs_sb = consts.tile([N_AGENTS, HID], f32)
        nc.sync.dma_start(hs_sb, hs)
        wmu_sb = consts.tile([128, 8, DS], f32)
        nc.sync.dma_start(wmu_sb, wmu.rearrange("(k p) m -> p k m", p=128))
        wvar_sb = consts.tile([128, 8, DS], f32)
        nc.sync.dma_start(wvar_sb, wvar.rearrange("(k p) m -> p k m", p=128))
        bmu_sb = consts.tile([DS, 1], f32)
        nc.sync.dma_start(bmu_sb, bmu)
        bvar_sb = consts.tile([DS, 1], f32)
        nc.sync.dma_start(bvar_sb, bvar)
        eps_sb = consts.tile([N_AGENTS, DS], f32)
        nc.sync.dma_start(eps_sb, eps)
        voc_sb = consts.tile([128, 32, DS], f32)
        nc.sync.dma_start(voc_sb, voc.rearrange("(j p) m -> p j m", p=128))

        # ---- transposes (PE) ---------------------------------------------
        # hidden_state (64, 1024) -> hsT (1024, 64) stored as (128, 8*64)
        hsT = consts.tile([128, 8 * N_AGENTS], f32)
        for k in range(8):
            pt = psT.tile([128, N_AGENTS], f32, tag="pt")
            nc.tensor.transpose(pt, hs_sb[:, 128 * k:128 * (k + 1)], ident[:N_AGENTS, :N_AGENTS])
            ev = nc.vector.tensor_copy if k % 2 == 0 else nc.scalar.copy
            ev(hsT[:, N_AGENTS * k:N_AGENTS * (k + 1)], pt)

        # vocab slice (4096, 32) -> vocab_T replicated to 4 quarters (128, 4096)
        vrep = consts.tile([128, V], f32)
        for j in range(32):
            pt2 = psT.tile([DS, 128], f32, tag="pt")
            nc.tensor.transpose(pt2, voc_sb[:, j, :], ident)
            ev = nc.vector.tensor_copy if j % 2 == 0 else nc.scalar.copy
            ev(vrep[0:DS, 128 * j:128 * (j + 1)], pt2)
        for c in range(1, 4):
            nc.sync.dma_start(vrep[DS * c:DS * (c + 1), :], vrep[0:DS, :])

        # ---- VIB head: muT, log_varT, outT (all (32, 64)) ----------------
        pmu = psMM.tile([DS, N_AGENTS], f32, tag="pmu")
        for k in range(8):
            nc.tensor.matmul(pmu, wmu_sb[:, k, :], hsT[:, N_AGENTS * k:N_AGENTS * (k + 1)],
                             start=(k == 0), stop=(k == 7))
        muT = consts.tile([DS, N_AGENTS], f32)
        nc.scalar.activation(muT, pmu, ACTF.Identity, bias=bmu_sb, scale=1.0)

        pvar = psMM.tile([DS, N_AGENTS], f32, tag="pvar")
        for k in range(8):
            nc.tensor.matmul(pvar, wvar_sb[:, k, :], hsT[:, N_AGENTS * k:N_AGENTS * (k + 1)],
                             start=(k == 0), stop=(k == 7))
        # std = exp(0.5 * (pvar + bvar)) = Exp(0.5 * pvar + 0.5 * bvar)
        bvar_half = consts.tile([DS, 1], f32)
        nc.scalar.mul(bvar_half, bvar_sb, 0.5)
        std = consts.tile([DS, N_AGENTS], f32)
        nc.scalar.activation(std, pvar, ACTF.Exp, bias=bvar_half, scale=0.5)

        peps = psT.tile([DS, N_AGENTS], f32, tag="pt")
        nc.tensor.transpose(peps, eps_sb, ident[:N_AGENTS, :N_AGENTS])
        epsT = consts.tile([DS, N_AGENTS], f32)
        nc.vector.tensor_copy(epsT, peps)

        # outT = epsT * std + muT  (32, 64): column = agent index.
        outT = consts.tile([DS, N_AGENTS], f32)
        tmp = consts.tile([DS, N_AGENTS], f32)
        nc.vector.tensor_mul(tmp, epsT, std)
        nc.vector.tensor_add(outT, tmp, muT)
        outTn = consts.tile([DS, N_AGENTS], f32)
        nc.vector.tensor_scalar_mul(outTn, outT, -1.0)
        # Quarter c of the 128-partition layout handles agents 16c..16c+16:
        # bias_{pos,neg}[32c+d, g] = (+/-)outT[d, 16c+g]
        bias_pos = consts.tile([128, 16], f32)
        bias_neg = consts.tile([128, 16], f32)
        nc.vector.tensor_copy(bias_pos[0:DS, :], outT[:, 0:16])
        nc.vector.tensor_copy(bias_neg[0:DS, :], outTn[:, 0:16])
        for c in range(1, 4):
            nc.sync.dma_start(bias_pos[DS * c:DS * (c + 1), :], outT[:, 16 * c:16 * (c + 1)])
            nc.sync.dma_start(bias_neg[DS * c:DS * (c + 1), :], outTn[:, 16 * c:16 * (c + 1)])

        # ---- VQ nearest-neighbour min over the vocab ---------------------
        fda = FDA if USE_CUSTOM else V
        mins = consts.tile([128, 16], f32)
        for g in range(16):  # agent group: quarter c handles agent 16*c + g
            sq_t = sqpool.tile([128, fda], bf16, tag="sq")
            nc.scalar.activation(sq_t, vrep[:, 0:fda], ACTF.Square,
                                 bias=bias_neg[:, g:g + 1], scale=1.0)
            cur, w = sq_t, fda
            while w > TREE_STOP_W:
                w //= 2
                nxt = tpool.tile([128, w], bf16, tag=f"lvl{w}")
                nc.vector.tensor_tensor(nxt, cur[:, 0:w], cur[:, w:2 * w], op=ALU.min)
                cur = nxt
            if USE_CUSTOM:
                tmin = tpool.tile([128, 1], f32, tag="tmin")
                nc.vector.tensor_reduce(tmin, cur, axis=mybir.AxisListType.X, op=ALU.min)
                scr = tpool.tile([128, V - fda], f32, tag="scr")
                nc.vector._custom_dve(
                    custom_op, out=scr, in0=vrep[:, fda:V],
                    s0=bias_pos[:, g:g + 1], s1=tmin, accum_out=mins[:, g:g + 1],
                )
            else:
                nc.vector.tensor_reduce(mins[:, g:g + 1], cur,
                                        axis=mybir.AxisListType.X, op=ALU.min)

        # ---- communication sum + output ----------------------------------
        part = consts.tile([128, 1], f32)
        nc.vector.tensor_reduce(part, mins, axis=mybir.AxisListType.X, op=ALU.add)
        qsum = [consts.tile([DS, 1], f32, name=f"qsum{i}", tag=f"qsum{i}") for i in range(3)]
        for c in range(1, 4):
            nc.sync.dma_start(qsum[c - 1], part[DS * c:DS * (c + 1), :])
        S = consts.tile([128, 1], f32)
        nc.vector.tensor_add(S[0:DS, :], part[0:DS, :], qsum[0])
        nc.vector.tensor_add(S[0:DS, :], S[0:DS, :], qsum[1])
        nc.vector.tensor_add(S[0:DS, :], S[0:DS, :], qsum[2])
        for c in range(1, 4):
            nc.sync.dma_start(S[DS * c:DS * (c + 1), :], S[0:DS, :])
        # comm = (mins - S) * -1 = S - token
        comm = consts.tile([128, 16], f32)
        nc.vector.tensor_scalar(comm, mins, S[:, 0:1], -1.0,
                                op0=ALU.subtract, op1=ALU.mult)
        pcomm = psT.tile([16, 128], f32, tag="pt")
        nc.tensor.transpose(pcomm, comm, ident)
        commT = consts.tile([16, 128], f32)
        nc.vector.tensor_copy(commT, pcomm)
        nc.sync.dma_start(out.rearrange("(c g) d -> g c d", c=4),
                          commT.rearrange("g (c d) -> g c d", c=4))

    nc.compile()
    return nc


def get_nc():
    global _NC
    if _NC is None:
        _NC = build_nc()
    return _NC


def make_in_maps(hidden_state, W_mu, b_mu, W_var, b_var, vocab, eps):
    def f(x):
        return np.ascontiguousarray(np.asarray(x, dtype=np.float32))

    in_maps = []
    for s in range(NCORES):
        sl = slice(DS * s, DS * (s + 1))
        in_maps.append({
            "hs": f(hidden_state),
            "wmu": f(np.asarray(W_mu)[:, sl]),
            "wvar": f(np.asarray(W_var)[:, sl]),
            "bmu": f(np.asarray(b_mu)[sl]).reshape(DS, 1),
            "bvar": f(np.asarray(b_var)[sl]).reshape(DS, 1),
            "eps": f(np.asarray(eps)[:, sl]),
            "voc": f(np.asarray(vocab)[:, sl]),
        })
    return in_maps


def kernel(hidden_state, W_mu, b_mu, W_var, b_var, vocab, eps):
    nc = get_nc()
    in_maps = make_in_maps(hidden_state, W_mu, b_mu, W_var, b_var, vocab, eps)
    res = run_bass_kernel_spmd(nc, in_maps, core_ids=list(range(NCORES)))
    return np.concatenate([r["out"] for r in res.results], axis=1).astype(np.float32)


# revision 7
# speedup vs baseline: 1.2183x; 1.2183x over previous
"""VQ-VIB agent-communication kernel for Trainium2 (8 NeuronCores).

Reference computation (all fp32):
    mu      = hidden_state @ W_mu + b_mu                  # (64, 256)
    log_var = hidden_state @ W_var + b_var                # (64, 256)
    out     = eps * exp(0.5 * log_var) + mu               # (64, 256)
    token[n, d] = min_v (out[n, d] - vocab[v, d])**2      # min over 4096
    comm[j] = sum_{i != j} token[i] = (sum_i token[i]) - token[j]

Sharding: the 256 comm dims are split 32-per-core across 8 cores. Each core
gets the full (transposed) hidden_state plus column slices of
W_mu/W_var/b_*/eps/vocab, computes its (64, 32) output slice independently
(no collectives), and the host concatenates along the dim axis. The host
passes hidden_state, eps and vocab pre-transposed — layout prep is part of
the sharding strategy and keeps the TensorE transpose work off the kernel's
critical path.

Per-core layout for the VQ min: SBUF partitions p = 32*c + d hold local dim d
replicated over four agent-quarters c; the free axis is the full 4096-entry
vocab. For each group g of four agents (one per quarter, agent = 16*c + g):
  - ScalarE computes Square(vocab_T + (-out)) into bf16 for vocab[0:FDA]
  - VectorE folds it with a bf16 tensor_tensor min tree + final reduce_min
  - a custom fused DVE op handles vocab[FDA:4096] in fp32:
        accum_out = min(seed, min_v (vocab_T[v] - out)^2)
    seeded with the tree result, giving the final per-(agent, dim) min.
"""

from contextlib import ExitStack

import numpy as np

import concourse.bass as bass
import concourse.tile as tile
from concourse import bacc, mybir
from concourse.bass_utils import run_bass_kernel_spmd
from concourse.masks import make_identity

N_AGENTS = 64
HID = 1024
D = 256
V = 4096
NCORES = 8
DS = D // NCORES  # 32 dims per core

# Tuning knobs
USE_CUSTOM = True   # fused sq-diff+min-reduce DVE op for the tail of the vocab
FDA = 3328          # vocab elements handled by the ScalarE square + bf16 min tree
TREE_STOP_W = 416   # switch from tensor_tensor min tree to reduce_min at this width
SQ_BUFS = 3

f32 = mybir.dt.float32
bf16 = mybir.dt.bfloat16
ALU = mybir.AluOpType
ACTF = mybir.ActivationFunctionType

_CUSTOM_OP = None
_NC = None


def _sqdiff_min_op():
    """Register (idempotently) the fused DVE op:
        out = (in0 - s0)^2 ; accum_out = min(s1, min_free(out))
    """
    global _CUSTOM_OP
    if _CUSTOM_OP is not None:
        return _CUSTOM_OP
    import concourse.dve_ops as dve_ops
    from concourse.dve_spec import C0, C1, AluOp, Spec, Src0, lower, sq
    from concourse.dve_uop import DveOpSpec

    name = "SQDIFF_MIN_ANT"
    for op in dve_ops.OPS:
        if op.name == name:
            _CUSTOM_OP = op
            return op

    def ref(in0, in1, s0, s1, imm2):
        b = ((in0.astype(np.float32) - s0) ** 2).astype(np.float32)
        p = b.shape[0]
        acc = np.minimum(
            np.broadcast_to(np.asarray(s1, np.float32).reshape(-1, 1), (p, 1)),
            b.reshape(p, -1).min(axis=-1, keepdims=True),
        ).astype(np.float32)
        return b, acc

    spec = Spec(body=sq(Src0 - C0), accum=AluOp.MIN, accum_init=C1, reference=ref)
    shas = {}
    for ver in ("v3", "v4"):
        shas[ver] = DveOpSpec(name=name, opcode=1, uops=lower(spec, ver=ver)).sha(ver)
    op = dve_ops.DveOp(name, spec, subdim=False, uops_sha=shas)
    dve_ops.OPS.append(op)
    dve_ops._SUB_OPCODE_FOR_NAME[name] = dve_ops._CUSTOM_DVE_ROW_BASE + len(dve_ops.OPS) - 1
    assert dve_ops._SUB_OPCODE_FOR_NAME[name] < 0x20
    dve_ops.CUSTOM_DVE_SPECS[name] = spec
    _CUSTOM_OP = op
    return op


def build_nc():
    """Trace the per-core Bass program (identical on all 8 cores)."""
    if USE_CUSTOM:
        custom_op = _sqdiff_min_op()
    nc = bacc.Bacc(
        "TRN2", target_bir_lowering=False, debug=False, enable_asserts=False,
        num_devices=NCORES,
    )
    hsT = nc.dram_tensor("hsT", (HID, N_AGENTS), f32, kind="ExternalInput").ap()
    wmu = nc.dram_tensor("wmu", (HID, DS), f32, kind="ExternalInput").ap()
    wvar = nc.dram_tensor("wvar", (HID, DS), f32, kind="ExternalInput").ap()
    bmu = nc.dram_tensor("bmu", (DS, 1), f32, kind="ExternalInput").ap()
    bvar = nc.dram_tensor("bvar", (DS, 1), f32, kind="ExternalInput").ap()
    epsT = nc.dram_tensor("epsT", (DS, N_AGENTS), f32, kind="ExternalInput").ap()
    vocT = nc.dram_tensor("vocT", (DS, V), f32, kind="ExternalInput").ap()
    out = nc.dram_tensor("out", (N_AGENTS, DS), f32, kind="ExternalOutput").ap()

    with tile.TileContext(nc) as tc, ExitStack() as ctx:
        consts = ctx.enter_context(tc.tile_pool(name="consts", bufs=1))
        sqpool = ctx.enter_context(tc.tile_pool(name="sq", bufs=SQ_BUFS))
        tpool = ctx.enter_context(tc.tile_pool(name="tree", bufs=2))
        psT = ctx.enter_context(tc.tile_pool(name="psT", bufs=2, space="PSUM"))
        psMM = ctx.enter_context(tc.tile_pool(name="psMM", bufs=1, space="PSUM"))

        ident = consts.tile([128, 128], f32)
        make_identity(nc, ident)

        # ---- input loads (all independent, parallel HWDGE queues) --------
        hsT_sb = consts.tile([128, 8, N_AGENTS], f32)
        nc.sync.dma_start(hsT_sb, hsT.rearrange("(k p) n -> p k n", p=128))
        wmu_sb = consts.tile([128, 8, DS], f32)
        nc.sync.dma_start(wmu_sb, wmu.rearrange("(k p) m -> p k m", p=128))
        wvar_sb = consts.tile([128, 8, DS], f32)
        nc.sync.dma_start(wvar_sb, wvar.rearrange("(k p) m -> p k m", p=128))
        bmu_sb = consts.tile([DS, 1], f32)
        nc.sync.dma_start(bmu_sb, bmu)
        bvar_sb = consts.tile([DS, 1], f32)
        nc.sync.dma_start(bvar_sb, bvar)
        epsT_sb = consts.tile([DS, N_AGENTS], f32)
        nc.sync.dma_start(epsT_sb, epsT)
        # vocab_T replicated into all 4 agent-quarters straight from HBM
        vrep = consts.tile([128, V], f32)
        for c in range(4):
            nc.sync.dma_start(vrep[DS * c:DS * (c + 1), :], vocT)

        # ---- VIB head: muT, log_varT, outT (all (32, 64)) ----------------
        pmu = psMM.tile([DS, N_AGENTS], f32, tag="pmu")
        for k in range(8):
            nc.tensor.matmul(pmu, wmu_sb[:, k, :], hsT_sb[:, k, :],
                             start=(k == 0), stop=(k == 7))
        muT = consts.tile([DS, N_AGENTS], f32)
        nc.scalar.activation(muT, pmu, ACTF.Identity, bias=bmu_sb, scale=1.0)

        pvar = psMM.tile([DS, N_AGENTS], f32, tag="pvar")
        for k in range(8):
            nc.tensor.matmul(pvar, wvar_sb[:, k, :], hsT_sb[:, k, :],
                             start=(k == 0), stop=(k == 7))
        # std = exp(0.5 * (pvar + bvar)) = Exp(0.5 * pvar + 0.5 * bvar)
        bvar_half = consts.tile([DS, 1], f32)
        nc.scalar.mul(bvar_half, bvar_sb, 0.5)
        std = consts.tile([DS, N_AGENTS], f32)
        nc.scalar.activation(std, pvar, ACTF.Exp, bias=bvar_half, scale=0.5)

        # outT = epsT * std + muT  (32, 64): column = agent index.
        outT = consts.tile([DS, N_AGENTS], f32)
        tmp = consts.tile([DS, N_AGENTS], f32)
        nc.vector.tensor_mul(tmp, epsT_sb, std)
        nc.vector.tensor_add(outT, tmp, muT)
        # Quarter c of the 128-partition layout handles agents 16c..16c+16:
        # bias_{pos,neg}[32c+d, g] = (+/-)outT[d, 16c+g]. PE matmul outputs
        # must start at PSUM partition 0/32/64, so build the transposed form
        # (16, 128) with four block transposes (free-offset placement is
        # unrestricted), then transpose back.
        pbT = psMM.tile([16, 128], f32, tag="pbT")
        for c in range(4):
            nc.tensor.transpose(pbT[:, DS * c:DS * (c + 1)],
                                outT[:, 16 * c:16 * (c + 1)], ident[0:DS, 0:DS])
        sbT = consts.tile([16, 128], f32)
        nc.vector.tensor_copy(sbT, pbT)
        pbias = psMM.tile([128, 16], f32, tag="pbias")
        nc.tensor.transpose(pbias, sbT, ident[0:16, 0:16])
        bias_pos = consts.tile([128, 16], f32)
        nc.vector.tensor_copy(bias_pos, pbias)
        bias_neg = consts.tile([128, 16], f32)
        nc.scalar.mul(bias_neg, pbias, -1.0)

        # ---- VQ nearest-neighbour min over the vocab ---------------------
        fda = FDA if USE_CUSTOM else V
        mins = consts.tile([128, 16], f32)
        for g in range(16):  # agent group: quarter c handles agent 16*c + g
            sq_t = sqpool.tile([128, fda], bf16, tag="sq")
            nc.scalar.activation(sq_t, vrep[:, 0:fda], ACTF.Square,
                                 bias=bias_neg[:, g:g + 1], scale=1.0)
            cur, w = sq_t, fda
            while w > TREE_STOP_W:
                w //= 2
                nxt = tpool.tile([128, w], bf16, tag=f"lvl{w}")
                nc.vector.tensor_tensor(nxt, cur[:, 0:w], cur[:, w:2 * w], op=ALU.min)
                cur = nxt
            if USE_CUSTOM:
                tmin = tpool.tile([128, 1], f32, tag="tmin")
                nc.vector.tensor_reduce(tmin, cur, axis=mybir.AxisListType.X, op=ALU.min)
                scr = tpool.tile([128, V - fda], f32, tag="scr")
                nc.vector._custom_dve(
                    custom_op, out=scr, in0=vrep[:, fda:V],
                    s0=bias_pos[:, g:g + 1], s1=tmin, accum_out=mins[:, g:g + 1],
                )
            else:
                nc.vector.tensor_reduce(mins[:, g:g + 1], cur,
                                        axis=mybir.AxisListType.X, op=ALU.min)

        # ---- communication sum + output ----------------------------------
        # M4[k, m] = 1 iff k % 32 == m % 32: one matmul turns the per-quarter
        # partial sums into the full agent sum replicated to every quarter.
        m4 = consts.tile([128, 128], f32)
        nc.gpsimd.memset(m4, 0.0)
        for off in range(-96, 97, 32):
            nc.gpsimd.affine_select(
                out=m4, in_=m4, compare_op=ALU.not_equal, fill=1.0,
                base=off, pattern=[[-1, 128]], channel_multiplier=1,
            )
        part = consts.tile([128, 1], f32)
        nc.vector.tensor_reduce(part, mins, axis=mybir.AxisListType.X, op=ALU.add)
        pS = psMM.tile([128, 1], f32, tag="pS")
        nc.tensor.matmul(pS, m4, part, start=True, stop=True)
        S = consts.tile([128, 1], f32)
        nc.scalar.copy(S, pS)
        # comm = (mins - S) * -1 = S - token
        comm = consts.tile([128, 16], f32)
        nc.vector.tensor_scalar(comm, mins, S[:, 0:1], -1.0,
                                op0=ALU.subtract, op1=ALU.mult)
        pcomm = psT.tile([16, 128], f32, tag="pt")
        nc.tensor.transpose(pcomm, comm, ident)
        commT = consts.tile([16, 128], f32)
        nc.vector.tensor_copy(commT, pcomm)
        nc.sync.dma_start(out.rearrange("(c g) d -> g c d", c=4),
                          commT.rearrange("g (c d) -> g c d", c=4))

    nc.compile()
    return nc


def get_nc():
    global _NC
    if _NC is None:
        _NC = build_nc()
    return _NC


def make_in_maps(hidden_state, W_mu, b_mu, W_var, b_var, vocab, eps):
    def f(x):
        return np.ascontiguousarray(np.asarray(x, dtype=np.float32))

    hsT = f(np.asarray(hidden_state).T)            # (1024, 64), shared
    epsT_full = np.asarray(eps).T                  # (256, 64)
    vocT_full = np.asarray(vocab).T                # (256, 4096)
    in_maps = []
    for s in range(NCORES):
        sl = slice(DS * s, DS * (s + 1))
        in_maps.append({
            "hsT": hsT,
            "wmu": f(np.asarray(W_mu)[:, sl]),
            "wvar": f(np.asarray(W_var)[:, sl]),
            "bmu": f(np.asarray(b_mu)[sl]).reshape(DS, 1),
            "bvar": f(np.asarray(b_var)[sl]).reshape(DS, 1),
            "epsT": f(epsT_full[sl]),
            "vocT": f(vocT_full[sl]),
        })
    return in_maps


def kernel(hidden_state, W_mu, b_mu, W_var, b_var, vocab, eps):
    nc = get_nc()
    in_maps = make_in_maps(hidden_state, W_mu, b_mu, W_var, b_var, vocab, eps)
    res = run_bass_kernel_spmd(nc, in_maps, core_ids=list(range(NCORES)))
    return np.concatenate([r["out"] for r in res.results], axis=1).astype(np.float32)
